# revision 1
# baseline (speedup 1.0000x reference)
"""Trainium2 Bass kernel for the projectile-integration environment.

Math (reference semantics):
    idx = [0, 0, 1, ..., K-2]           (f shifted right by one, f[0] repeated)
    a_k = (DT/M) * f[idx_k] - DT*G*e3
    v_k = v_0 + cumsum(a)_k
    p_k = p_0 + (DT/2) * cumsum(v + v_prev)_k
        = p_0 + (DT/2)*v_0 + DT*cumsum(v)_k - (DT/2)*v_k

Two chained prefix sums over K = 8M rows x 3 channels. Parallelization:
the sequence is cut into blocks of W rows (one block per SBUF partition
per tile per core). The host computes, in float64, the exact exclusive
prefix carried into every block for both cumsum levels (VOFF for v, PB
for p) — a cheap O(K) reduction. Each NeuronCore then processes its
shard fully independently: per 128-partition tile it runs the native
vector-engine prefix-scan (tensor_tensor_scan) along the free dim to get
within-block cumsums, and applies the per-block affine offsets with
scalar-engine activations. Gravity is folded into the first scan via the
scan's second data operand (a constant -M*G tile on the z channel).

No collectives, no cross-tile serialization: every tile is independent.
Per-core HBM traffic is the minimum possible (read f shard once, write
v and p shards once).
"""

import os
import sys

for _p in ("/opt/trn_rl_repo",):
    if _p not in sys.path and os.path.isdir(_p):
        sys.path.insert(0, _p)

import numpy as np

import concourse.bass as bass  # noqa: F401
import concourse.mybir as mybir
from concourse import bacc
from concourse.bass_utils import run_bass_kernel_spmd
from concourse.tile import TileContext

DT = 0.01
G = 9.81
M = 1.5

K = 8388608
NCORES = 8
P = 128          # SBUF partitions
W = 1024         # rows per partition per tile (= block size)
L = K // NCORES  # rows per core
R = P * W        # rows per tile
NT = L // R      # tiles per core


def build_bass(L_=L, W_=W):
    """Build the per-core SPMD Bass module. Identical program on all cores;
    all per-core differences come in through the input tensors."""
    P_ = 128
    R_ = P_ * W_
    nt = L_ // R_
    assert nt * R_ == L_

    f32 = mybir.dt.float32
    add = mybir.AluOpType.add
    mult = mybir.AluOpType.mult
    ident = mybir.ActivationFunctionType.Identity

    nc = bacc.Bacc(None, target_bir_lowering=False)
    fs = nc.dram_tensor("fs", [L_, 3], f32, kind="ExternalInput")
    voff = nc.dram_tensor("voff", [P_, nt * 3], f32, kind="ExternalInput")
    pb = nc.dram_tensor("pb", [P_, nt * 3], f32, kind="ExternalInput")
    v_out = nc.dram_tensor("v", [L_, 3], f32, kind="ExternalOutput")
    p_out = nc.dram_tensor("p", [L_, 3], f32, kind="ExternalOutput")

    # [NT, 128, W, 3]: tile i, partition p holds rows [i*R + p*W, i*R + (p+1)*W)
    fs_t = fs.rearrange("(i p w) c -> i p w c", p=P_, w=W_)
    v_t = v_out.rearrange("(i p w) c -> i p w c", p=P_, w=W_)
    p_t = p_out.rearrange("(i p w) c -> i p w c", p=P_, w=W_)

    with TileContext(nc) as tc:
        with (
            tc.tile_pool(name="const", bufs=1) as cpool,
            tc.tile_pool(name="fin", bufs=3) as fpool,
            tc.tile_pool(name="u", bufs=2) as upool,
            tc.tile_pool(name="vv", bufs=3) as vpool,
            tc.tile_pool(name="s", bufs=2) as spool,
            tc.tile_pool(name="pp", bufs=3) as ppool,
        ):
            zero = cpool.tile([P_, W_], f32)
            gz = cpool.tile([P_, W_], f32)
            nc.vector.memset(zero[:], 0.0)
            nc.vector.memset(gz[:], -M * G)
            voffs = cpool.tile([P_, nt * 3], f32)
            pbs = cpool.tile([P_, nt * 3], f32)
            nc.sync.dma_start(out=voffs[:], in_=voff[:])
            nc.sync.dma_start(out=pbs[:], in_=pb[:])
            d1 = (zero, zero, gz)

            for i in range(nt):
                ft = fpool.tile([P_, W_, 3], f32)
                nc.sync.dma_start(out=ft[:], in_=fs_t[i])
                ut = upool.tile([P_, W_, 3], f32)
                vt = vpool.tile([P_, W_, 3], f32)
                st = spool.tile([P_, W_, 3], f32)
                pt = ppool.tile([P_, W_, 3], f32)
                for c in range(3):
                    # u = within-partition cumsum of (f + (-M*G on z))
                    nc.vector.tensor_tensor_scan(
                        out=ut[:, :, c], data0=ft[:, :, c], data1=d1[c][:],
                        initial=0.0, op0=add, op1=add,
                    )
                for c in range(3):
                    # v = (DT/M)*u + VOFF[block]
                    nc.scalar.activation(
                        out=vt[:, :, c], in_=ut[:, :, c], func=ident,
                        bias=voffs[:, i * 3 + c : i * 3 + c + 1], scale=DT / M,
                    )
                for c in range(3):
                    # s = within-partition cumsum of v
                    nc.vector.tensor_tensor_scan(
                        out=st[:, :, c], data0=vt[:, :, c], data1=zero[:],
                        initial=0.0, op0=add, op1=add,
                    )
                for c in range(3):
                    # ptmp = DT*s + PB[block]
                    nc.scalar.activation(
                        out=pt[:, :, c], in_=st[:, :, c], func=ident,
                        bias=pbs[:, i * 3 + c : i * 3 + c + 1], scale=DT,
                    )
                for c in range(3):
                    # p = ptmp - (DT/2)*v
                    nc.vector.scalar_tensor_tensor(
                        out=pt[:, :, c], in0=vt[:, :, c], scalar=-DT / 2,
                        in1=pt[:, :, c], op0=mult, op1=add,
                    )
                nc.sync.dma_start(out=v_t[i], in_=vt[:])
                nc.sync.dma_start(out=p_t[i], in_=pt[:])
    nc.finalize()
    return nc


def host_prepare(f, p_0, v_0, ncores=NCORES, W_=W):
    """Host-side (float64) per-block exclusive-prefix offsets + shard packing.

    Returns in_maps (one dict per core). Block m covers rows [m*W, (m+1)*W).
    Per core, blocks are laid out [nt, 128] (tile-major, then partition).
    """
    f = np.asarray(f)
    K_ = f.shape[0]
    L_ = K_ // ncores
    NB = K_ // W_
    nt = L_ // (128 * W_)
    p0 = np.asarray(p_0, np.float64)
    v0 = np.asarray(v_0, np.float64)
    e3 = np.array([0.0, 0.0, 1.0])

    # shifted f (f[0] repeated), float32 — identical bits to what device sees
    fs32 = np.empty((K_, 3), np.float32)
    fs32[0] = f[0]
    fs32[1:] = f[:-1]

    blocks = fs32.reshape(NB, W_, 3)
    bs = blocks.sum(axis=1, dtype=np.float64)                 # block sums of fs
    wvec = np.arange(W_, 0, -1, dtype=np.float64)             # weight W-t
    wbs = np.einsum("bwc,w->bc", blocks, wvec, dtype=np.float64)
    EU = np.zeros((NB, 3))
    np.cumsum(bs[:-1], axis=0, out=EU[1:])                    # excl prefix of fs
    m_arr = np.arange(NB, dtype=np.float64)[:, None]
    VOFF = v0[None, :] + (DT / M) * EU - (m_arr * W_) * DT * G * e3[None, :]
    # sum of v over block m (float64, analytic)
    sv = (
        W_ * v0[None, :]
        + (DT / M) * (W_ * EU + wbs)
        - DT * G * e3[None, :] * (W_ * (m_arr * W_) + W_ * (W_ + 1) / 2.0)
    )
    EV = np.zeros((NB, 3))
    np.cumsum(sv[:-1], axis=0, out=EV[1:])                    # excl prefix of v
    PB = DT * EV + p0[None, :] + (DT / 2) * v0[None, :]

    # pack [NB,3] -> per-core [128, nt*3], voff_packed[p, i*3+c] = block (i*128+p)
    def pack(X):
        Xc = X.astype(np.float32).reshape(ncores, nt, 128, 3)
        return np.ascontiguousarray(Xc.transpose(0, 2, 1, 3).reshape(ncores, 128, nt * 3))

    vp = pack(VOFF)
    pbp = pack(PB)
    return [
        {"fs": fs32[s * L_ : (s + 1) * L_], "voff": vp[s], "pb": pbp[s]}
        for s in range(ncores)
    ]


_NC = None
LAST_RESULTS = None  # BassKernelResults of the most recent run (for profiling)


def _get_nc():
    global _NC
    if _NC is None:
        _NC = build_bass()
    return _NC


def kernel(f, p_0, v_0):
    global LAST_RESULTS
    f = np.asarray(f, np.float32)
    in_maps = host_prepare(f, p_0, v_0)
    nc = _get_nc()
    res = run_bass_kernel_spmd(nc, in_maps, core_ids=list(range(NCORES)))
    LAST_RESULTS = res
    v = np.concatenate([r["v"] for r in res.results], axis=0)
    p = np.concatenate([r["p"] for r in res.results], axis=0)
    return p, v



# revision 3
# speedup vs baseline: 1.9376x; 1.9376x over previous
"""Trainium2 Bass kernel for the projectile-integration environment.

Math (reference semantics, 0-based k):
    fs = f shifted right one (f[0] repeated)
    v[k] = v_0 + (DT/M)*cumsum(fs)[k] - DT*G*(k+1)*e3
    p[k] = p_0 + (DT/2)*v_0 + DT*cumsum(v)[k] - (DT/2)*v[k]

Formulation: cut the sequence into blocks of B=128 rows and lay each
block along the 128 SBUF partitions (block index = free dim). Then both
chained prefix sums are matmuls with precomputed triangular stationary
matrices on the otherwise-idle Tensor engine:

    v[t,j] = sum_{p<=t} alpha * xhat[p,j]                  (W1 = alpha*tri)
    p[t,j] = sum_{p<=t} DT*alpha*(t-p+1/2) * xhat[p,j] + PB_j

where xhat = fs + (-M*G folded into every z element: yields the
-DT*G*(t+1) gravity ramp through the cumsum) + (VOFF_j/alpha spike on
each block's first element: the cumsum then carries the per-block
velocity offset VOFF_j with exactly the right t-weighting in both
outputs). PB_j (per-block position offset) enters via a third, 1-row
accumulate-matmul into the p PSUM bank. The per-block exclusive
prefixes VOFF/PB are computed on the host in float64 (cheap O(K)
reduction), exactly as in the scan-based variant.

All device I/O is bf16 (tolerance is 2e-2; the large magnitudes ride
in the f64 host offsets): 18.9MB per core instead of 36.7MB f32.
Vector/scalar engines only do PSUM->SBUF bf16-convert copies.
"""

import os
import sys

for _p in ("/opt/trn_rl_repo",):
    if _p not in sys.path and os.path.isdir(_p):
        sys.path.insert(0, _p)

import ml_dtypes
import numpy as np

import concourse.bass as bass  # noqa: F401
import concourse.mybir as mybir
from concourse import bacc
from concourse.bass_utils import run_bass_kernel_spmd
from concourse.tile import TileContext

DT = 0.01
G = 9.81
M = 1.5
ALPHA = DT / M

K = 8388608
NCORES = 8
B = 128               # block size = partition count
L = K // NCORES       # rows per core
NBC = L // B          # blocks per core (8192)
F = 3 * NBC           # free columns per core (channel-major planes)

CH = 512              # matmul chunk = one PSUM bank of f32
XT = 2048             # input-tile columns
OT = 4096             # output-tile columns (half a channel plane)

BF16 = ml_dtypes.bfloat16


def build_bass():
    """Per-core SPMD Bass module (identical on all cores)."""
    f32 = mybir.dt.float32
    bf16 = mybir.dt.bfloat16

    nc = bacc.Bacc(None, target_bir_lowering=False)
    x = nc.dram_tensor("x", [B, F], bf16, kind="ExternalInput")
    pbt = nc.dram_tensor("pbt", [1, F], bf16, kind="ExternalInput")
    w1t = nc.dram_tensor("w1t", [B, B], bf16, kind="ExternalInput")
    w2t = nc.dram_tensor("w2t", [B, B], bf16, kind="ExternalInput")
    wbt = nc.dram_tensor("wbt", [1, B], bf16, kind="ExternalInput")
    v_out = nc.dram_tensor("v", [B, F], bf16, kind="ExternalOutput")
    p_out = nc.dram_tensor("p", [B, F], bf16, kind="ExternalOutput")

    n_xt = F // XT

    with TileContext(nc) as tc:
        with (
            tc.tile_pool(name="w", bufs=1) as wpool,
            tc.tile_pool(name="xin", bufs=1) as xpool,
            tc.tile_pool(name="vo", bufs=2) as vopool,
            tc.tile_pool(name="po", bufs=2) as popool,
            tc.psum_pool(name="vps", bufs=3) as vpsp,
            tc.psum_pool(name="pps", bufs=3) as ppsp,
        ):
            w1s = wpool.tile([B, B], bf16)
            nc.sync.dma_start(out=w1s[:], in_=w1t[:])
            w2s = wpool.tile([B, B], bf16)
            nc.sync.dma_start(out=w2s[:], in_=w2t[:])
            wbs = wpool.tile([1, B], bf16)
            nc.sync.dma_start(out=wbs[:], in_=wbt[:])
            pbs = wpool.tile([1, F], bf16)
            nc.sync.dma_start(out=pbs[:], in_=pbt[:])

            # Pre-issue every input DMA: the whole input (48KB/partition)
            # sits in SBUF, so the inbound stream never stalls behind
            # compute waits on the Sync engine.
            xts = []
            for i in range(n_xt):
                xt_ = xpool.tile([B, XT], bf16, name=f"xt{i}")
                nc.sync.dma_start(out=xt_[:], in_=x[:, i * XT : (i + 1) * XT])
                xts.append(xt_)

            for h in range(F // OT):
                vo = vopool.tile([B, OT], bf16)
                po = popool.tile([B, OT], bf16)
                for i in range(OT // XT):
                    xt_ = xts[h * (OT // XT) + i]
                    for q in range(XT // CH):
                        off = q * CH
                        g = h * OT + i * XT + off   # global column
                        lo = i * XT + off           # column within out tile
                        vps = vpsp.tile([B, CH], f32)
                        pps = ppsp.tile([B, CH], f32)
                        nc.tensor.matmul(
                            out=vps[:], lhsT=w1s[:], rhs=xt_[:, off : off + CH],
                            start=True, stop=True,
                        )
                        nc.tensor.matmul(
                            out=pps[:], lhsT=w2s[:], rhs=xt_[:, off : off + CH],
                            start=True, stop=False,
                        )
                        nc.tensor.matmul(
                            out=pps[:], lhsT=wbs[:], rhs=pbs[:, g : g + CH],
                            start=False, stop=True,
                        )
                        nc.scalar.copy(out=vo[:, lo : lo + CH], in_=vps[:])
                        nc.vector.tensor_copy(out=po[:, lo : lo + CH], in_=pps[:])
                nc.sync.dma_start(out=v_out[:, h * OT : (h + 1) * OT], in_=vo[:])
                nc.sync.dma_start(out=p_out[:, h * OT : (h + 1) * OT], in_=po[:])
    nc.finalize()
    return nc


def host_prepare(f, p_0, v_0):
    """Float64 per-block exclusive-prefix offsets + bf16 plane packing."""
    f = np.asarray(f, np.float32)
    p0 = np.asarray(p_0, np.float64)
    v0 = np.asarray(v_0, np.float64)
    e3 = np.array([0.0, 0.0, 1.0])
    NB = K // B

    fs = np.empty((K, 3), np.float32)
    fs[0] = f[0]
    fs[1:] = f[:-1]

    blocks = fs.reshape(NB, B, 3)
    bs = blocks.sum(axis=1, dtype=np.float64)
    wvec = np.arange(B, 0, -1, dtype=np.float64)
    wbs_ = np.einsum("bwc,w->bc", blocks, wvec, dtype=np.float64)
    EU = np.zeros((NB, 3))
    np.cumsum(bs[:-1], axis=0, out=EU[1:])
    m_arr = np.arange(NB, dtype=np.float64)[:, None]
    VOFF = v0[None, :] + ALPHA * EU - (m_arr * B) * DT * G * e3[None, :]
    sv = (
        B * v0[None, :]
        + ALPHA * (B * EU + wbs_)
        - DT * G * e3[None, :] * (B * (m_arr * B) + B * (B + 1) / 2.0)
    )
    EV = np.zeros((NB, 3))
    np.cumsum(sv[:-1], axis=0, out=EV[1:])
    PB = DT * EV + p0[None, :] + (DT / 2) * v0[None, :]

    # xhat = fs + gravity fold (z, every row) + VOFF/alpha spike (row 0)
    xhat = blocks.copy()
    xhat[:, :, 2] += np.float32(-M * G)
    xhat[:, 0, :] += (VOFF / ALPHA).astype(np.float32)

    # stationary weights (lhsT layout: [p, t])
    tt = np.arange(B, dtype=np.float64)
    pp = tt[:, None]
    mask = pp <= tt[None, :]
    w1 = np.where(mask, ALPHA, 0.0).astype(BF16)
    w2 = np.where(mask, DT * ALPHA * (tt[None, :] - pp + 0.5), 0.0).astype(BF16)
    wb = np.ones((1, B), BF16)

    in_maps = []
    for s in range(NCORES):
        xc = xhat[s * NBC : (s + 1) * NBC]          # [NBC, B, 3]
        xplane = np.ascontiguousarray(
            xc.transpose(1, 2, 0).reshape(B, F)      # [p, c*NBC + j]
        ).astype(BF16)
        pbc = PB[s * NBC : (s + 1) * NBC]            # [NBC, 3]
        pbplane = np.ascontiguousarray(pbc.T.reshape(1, F)).astype(BF16)
        in_maps.append(
            {"x": xplane, "pbt": pbplane, "w1t": w1, "w2t": w2, "wbt": wb}
        )
    return in_maps


_NC = None
LAST_RESULTS = None  # BassKernelResults of the most recent run (for profiling)


def _get_nc():
    global _NC
    if _NC is None:
        _NC = build_bass()
    return _NC


def kernel(f, p_0, v_0):
    global LAST_RESULTS
    in_maps = host_prepare(f, p_0, v_0)
    nc = _get_nc()
    res = run_bass_kernel_spmd(nc, in_maps, core_ids=list(range(NCORES)))
    LAST_RESULTS = res

    def unpack(name):
        parts = []
        for r in res.results:
            plane = np.asarray(r[name]).reshape(B, 3, NBC)
            parts.append(
                plane.transpose(2, 0, 1).reshape(L, 3).astype(np.float32)
            )
        return np.concatenate(parts, axis=0)

    return unpack("p"), unpack("v")
